# revision 7
# baseline (speedup 1.0000x reference)
"""EventDrivenODECell Trainium2 kernel.

Math (reference semantics):
  dt = (t_end - t_start)/5
  5 Euler steps: h += dt * (W3 tanh(W2 tanh(W1a h + [bd1 + W1b te(t)])) + bd3)
    where te(t) depends only on the scalar t -> folded on host into a
    per-step bias  b1s = bd1 + W1b @ te(t_s);  dt folded into W3/bd3.
  event: out = h + sigmoid(Wg ef + bg) * (We2 relu(We1h h + We1e ef + be1) + be2)

Device layout: feature-major activations [feat, batch]; batch sharded 8 ways
(8192 rows/core), processed in 8 column-chunks of 1024 (= 2 PSUM banks per
[128,1024] psum tile; matmuls cover it as two N=512 halves). Matmul inputs
are float32r (full-rate PE mode, ~12-bit mantissa); the h accumulator is
stored f32r (rounding applied by the DVE update op); biases and PSUM stay
fp32. Weight blocks are kept stationary across 2 chunks x 2 N-halves
(4 matmuls per LDWEIGHTS). tanh/sigmoid on ACT (bias fused), relu and all
elementwise adds on DVE (bias fused via scalar_tensor_tensor/tensor_scalar).
"""

import os
import sys

sys.path.insert(0, "/opt/trn_rl_repo")

import numpy as np

import concourse.bacc as bacc
import concourse.mybir as mybir
import concourse.tile as tile
from concourse.bass_utils import run_bass_kernel_spmd

B = 65536
HID = 256
EVT = 64
TEMB = 32
NUM_STEPS = 5
N_CORES = 8
R = B // N_CORES          # rows per core
CHUNK = 1024
N_CHUNKS = R // CHUNK     # 8
GROUP = 2                 # chunks per compute group (2 chunks x 2 m-halves
                          # x 2 banks = all 8 PSUM banks per layer)

MODE = os.environ.get("KMODE", "f32r")   # "f32r" | "f32"

f32 = mybir.dt.float32
f32r = mybir.dt.float32r

_CACHE = {}

# bias-pack column indices
COL_B1S = 0          # 0..4: per-step layer-1 bias
COL_B2 = 5
COL_B3 = 6
COL_BE1 = 7
COL_BE2 = 8
COL_BG = 9
N_BIAS_COLS = 10


def _build(mode):
    wdt = f32r if mode == "f32r" else f32
    nc = bacc.Bacc("TRN2", target_bir_lowering=False, debug=False,
                   num_devices=N_CORES)

    hT_d = nc.dram_tensor("hT", [HID, R], wdt, kind="ExternalInput")
    efT_d = nc.dram_tensor("efT", [EVT, R], wdt, kind="ExternalInput")
    w1_d = nc.dram_tensor("w1", [HID, HID], wdt, kind="ExternalInput")
    w2_d = nc.dram_tensor("w2", [HID, HID], wdt, kind="ExternalInput")
    w3_d = nc.dram_tensor("w3", [HID, HID], wdt, kind="ExternalInput")
    we1h_d = nc.dram_tensor("we1h", [HID, HID], wdt, kind="ExternalInput")
    we1e_d = nc.dram_tensor("we1e", [EVT, HID], wdt, kind="ExternalInput")
    we2_d = nc.dram_tensor("we2", [HID, HID], wdt, kind="ExternalInput")
    wg_d = nc.dram_tensor("wg", [EVT, HID], wdt, kind="ExternalInput")
    biasp_d = nc.dram_tensor("biasp", [HID, N_BIAS_COLS], f32,
                             kind="ExternalInput")
    outT_d = nc.dram_tensor("outT", [HID, R], f32, kind="ExternalOutput")

    Tanh = mybir.ActivationFunctionType.Tanh
    Sigmoid = mybir.ActivationFunctionType.Sigmoid
    add = mybir.AluOpType.add
    mult = mybir.AluOpType.mult
    vmax = mybir.AluOpType.max

    with tile.TileContext(nc) as tc:
        with (
            tc.tile_pool(name="consts", bufs=1) as consts,
            tc.tile_pool(name="h", bufs=1) as h_pool,
            tc.tile_pool(name="z1", bufs=2 * GROUP + 2) as z1_pool,
            tc.tile_pool(name="z2", bufs=2 * GROUP + 2) as z2_pool,
            tc.tile_pool(name="efc", bufs=4) as ef_pool,
            tc.tile_pool(name="stage", bufs=6) as stage_pool,
            tc.tile_pool(name="psum", bufs=4, space="PSUM") as psum_pool,
        ):
            # ---- constants ----
            def load_w(d, name, kparts, kdim=128):
                ts = []
                for k in range(kparts):
                    t = consts.tile([kdim, HID], wdt, tag=f"{name}{k}",
                                    name=f"{name}{k}")
                    nc.sync.dma_start(t[:], d.ap()[k * kdim:(k + 1) * kdim, :])
                    ts.append(t)
                return ts

            w1 = load_w(w1_d, "w1", 2)
            w2 = load_w(w2_d, "w2", 2)
            w3 = load_w(w3_d, "w3", 2)
            we1h = load_w(we1h_d, "we1h", 2)
            we2 = load_w(we2_d, "we2", 2)
            we1e = load_w(we1e_d, "we1e", 1, kdim=EVT)[0]   # [64, 256]
            wg = load_w(wg_d, "wg", 1, kdim=EVT)[0]         # [64, 256]

            biasp = []
            for m in range(2):
                t = consts.tile([128, N_BIAS_COLS], f32, tag=f"biasp{m}",
                                name=f"biasp{m}")
                nc.sync.dma_start(t[:], biasp_d.ap()[m * 128:(m + 1) * 128, :])
                biasp.append(t)

            def bcol(m, col):
                return biasp[m][:, col:col + 1]

            # ---- persistent h tiles (f32r in fast mode) ----
            h = [[h_pool.tile([128, CHUNK], wdt, tag=f"h{c}_{m}",
                              name=f"h{c}_{m}")
                  for m in range(2)] for c in range(N_CHUNKS)]
            for c in range(N_CHUNKS):
                for m in range(2):
                    nc.sync.dma_start(
                        h[c][m][:],
                        hT_d.ap()[m * 128:(m + 1) * 128,
                                  c * CHUNK:(c + 1) * CHUNK])

            def matmul_layer(chunks, win, xs, psums, kparts=2, extra=None):
                """psums[c][m] ([128,CHUNK] PSUM) = win.T @ xs[c] (+ extra).

                Weight block (k,m) stays stationary across len(chunks)*2
                N=512 matmuls. extra = (w_tile[EVT,HID], {c: x_ap}) adds an
                EVT-dim contribution.
                """
                n_acc = kparts + (1 if extra is not None else 0)
                for m in range(2):
                    for k in range(kparts):
                        wblk = win[k][:, m * 128:(m + 1) * 128]
                        for c in chunks:
                            for nh in range(CHUNK // 512):
                                nsl = slice(nh * 512, (nh + 1) * 512)
                                nc.tensor.matmul(
                                    psums[c][m][:, nsl],
                                    wblk, xs[c][k][:, nsl],
                                    start=(k == 0), stop=(k == n_acc - 1))
                    if extra is not None:
                        ew, exs = extra
                        eblk = ew[:, m * 128:(m + 1) * 128]
                        for c in chunks:
                            for nh in range(CHUNK // 512):
                                nsl = slice(nh * 512, (nh + 1) * 512)
                                nc.tensor.matmul(
                                    psums[c][m][:, nsl], eblk,
                                    exs[c][:, nsl],
                                    start=False, stop=True)

            def act_layer(chunks, psums, pool, act, bias_col, out_dt):
                outs = {}
                for c in chunks:
                    ts = []
                    for m in range(2):
                        o = pool.tile([128, CHUNK], out_dt, tag="z",
                                      name=f"z{c}_{m}")
                        nc.scalar.activation(o[:], psums[c][m][:], act,
                                             bias=bcol(m, bias_col))
                        ts.append(o)
                    outs[c] = ts
                return outs

            def new_psums(chunks):
                return {c: [psum_pool.tile([128, CHUNK], f32, tag="ps",
                                           name=f"ps{c}_{m}")
                            for m in range(2)] for c in chunks}

            groups = [range(g * GROUP, (g + 1) * GROUP)
                      for g in range(N_CHUNKS // GROUP)]

            for chunks in groups:
                # ---- ODE: 5 Euler steps, layer-major within the group ----
                for s in range(NUM_STEPS):
                    ps1 = new_psums(chunks)
                    matmul_layer(chunks, w1, {c: h[c] for c in chunks}, ps1)
                    z1 = act_layer(chunks, ps1, z1_pool, Tanh,
                                   COL_B1S + s, wdt)
                    ps2 = new_psums(chunks)
                    matmul_layer(chunks, w2, z1, ps2)
                    z2 = act_layer(chunks, ps2, z2_pool, Tanh, COL_B2, wdt)
                    ps3 = new_psums(chunks)
                    matmul_layer(chunks, w3, z2, ps3)
                    for c in chunks:
                        for m in range(2):
                            # h += (psum + b3)  (rounds h to wdt on store)
                            nc.vector.scalar_tensor_tensor(
                                h[c][m][:], ps3[c][m][:], bcol(m, COL_B3),
                                h[c][m][:], op0=add, op1=add)

                # ---- event update for this group ----
                efs = {}
                for c in chunks:
                    efc = ef_pool.tile([EVT, CHUNK], wdt, tag="ef",
                                       name=f"ef{c}")
                    nc.sync.dma_start(
                        efc[:], efT_d.ap()[:, c * CHUNK:(c + 1) * CHUNK])
                    efs[c] = efc[:]
                psu = new_psums(chunks)
                matmul_layer(chunks, we1h, {c: h[c] for c in chunks}, psu,
                             extra=(we1e, efs))
                u1 = {}
                for c in chunks:
                    ts = []
                    for m in range(2):
                        o = z1_pool.tile([128, CHUNK], wdt, tag="z",
                                         name=f"u{c}_{m}")
                        # relu(psum + be1) on DVE
                        nc.vector.tensor_scalar(
                            o[:], psu[c][m][:], bcol(m, COL_BE1), 0.0,
                            op0=add, op1=vmax)
                        ts.append(o)
                    u1[c] = ts
                # gate first (own PSUM phase), then upd — keeps each event
                # sub-phase within the 4 psum slots, no cross-phase cycle.
                psg = new_psums(chunks)
                for m in range(2):
                    gblk = wg[:, m * 128:(m + 1) * 128]
                    for c in chunks:
                        for nh in range(CHUNK // 512):
                            nsl = slice(nh * 512, (nh + 1) * 512)
                            nc.tensor.matmul(psg[c][m][:, nsl], gblk,
                                             efs[c][:, nsl],
                                             start=True, stop=True)
                gates = {}
                for c in chunks:
                    ts = []
                    for m in range(2):
                        gate = z2_pool.tile([128, CHUNK], f32, tag="z",
                                            name=f"g{c}_{m}")
                        nc.scalar.activation(gate[:], psg[c][m][:], Sigmoid,
                                             bias=bcol(m, COL_BG))
                        ts.append(gate)
                    gates[c] = ts
                psp = new_psums(chunks)
                matmul_layer(chunks, we2, u1, psp)
                for c in chunks:
                    for m in range(2):
                        # tmp = (psum_upd + be2) * gate
                        tmp = z2_pool.tile([128, CHUNK], f32, tag="z",
                                           name=f"t{c}_{m}")
                        nc.vector.scalar_tensor_tensor(
                            tmp[:], psp[c][m][:], bcol(m, COL_BE2),
                            gates[c][m][:], op0=add, op1=mult)
                        # out = tmp + h
                        stg = stage_pool.tile([128, CHUNK], f32, tag="st",
                                              name=f"s{c}_{m}")
                        nc.vector.tensor_add(stg[:], tmp[:], h[c][m][:])
                        nc.sync.dma_start(
                            outT_d.ap()[m * 128:(m + 1) * 128,
                                        c * CHUNK:(c + 1) * CHUNK],
                            stg[:])

    nc.finalize()
    return nc


def _get_nc(mode):
    if mode not in _CACHE:
        _CACHE[mode] = _build(mode)
    return _CACHE[mode]


LAST_RESULT = None


def kernel(h_prev, event_features, t_start, t_end,
           Wt1, bt1, Wt2, bt2,
           Wd1, bd1, Wd2, bd2, Wd3, bd3,
           We1, be1, We2, be2, Wg, bg):
    global LAST_RESULT
    assert h_prev.shape == (B, HID) and event_features.shape == (B, EVT)

    # ---- host-side folding (float64 for exactness, cast to f32) ----
    f8 = np.float64
    dt = (f8(t_end) - f8(t_start)) / NUM_STEPS
    b1s = np.empty((HID, NUM_STEPS), dtype=f8)
    for s in range(NUM_STEPS):
        t = f8(t_start) + s * dt
        te = np.tanh(t * Wt1[:, 0].astype(f8) + bt1.astype(f8))
        te = Wt2.astype(f8) @ te + bt2.astype(f8)
        b1s[:, s] = bd1.astype(f8) + Wd1[:, HID:].astype(f8) @ te

    w1T = np.ascontiguousarray(Wd1[:, :HID].T, dtype=np.float32)
    w2T = np.ascontiguousarray(Wd2.T, dtype=np.float32)
    w3T = np.ascontiguousarray((dt * Wd3.astype(f8)).T.astype(np.float32))
    we1hT = np.ascontiguousarray(We1[:, :HID].T, dtype=np.float32)
    we1eT = np.ascontiguousarray(We1[:, HID:].T, dtype=np.float32)
    we2T = np.ascontiguousarray(We2.T, dtype=np.float32)
    wgT = np.ascontiguousarray(Wg.T, dtype=np.float32)

    biasp = np.zeros((HID, N_BIAS_COLS), dtype=f8)
    biasp[:, COL_B1S:COL_B1S + NUM_STEPS] = b1s
    biasp[:, COL_B2] = bd2.astype(f8)
    biasp[:, COL_B3] = dt * bd3.astype(f8)
    biasp[:, COL_BE1] = be1.astype(f8)
    biasp[:, COL_BE2] = be2.astype(f8)
    biasp[:, COL_BG] = bg.astype(f8)
    biasp = biasp.astype(np.float32)

    hT = np.ascontiguousarray(h_prev.T, dtype=np.float32)      # [HID, B]
    efT = np.ascontiguousarray(event_features.T, dtype=np.float32)

    shared = dict(w1=w1T, w2=w2T, w3=w3T, we1h=we1hT, we1e=we1eT,
                  we2=we2T, wg=wgT, biasp=biasp)
    in_maps = []
    for c in range(N_CORES):
        sl = slice(c * R, (c + 1) * R)
        in_maps.append(dict(
            hT=np.ascontiguousarray(hT[:, sl]),
            efT=np.ascontiguousarray(efT[:, sl]),
            **shared))

    nc = _get_nc(MODE)
    res = run_bass_kernel_spmd(nc, in_maps, core_ids=list(range(N_CORES)))
    LAST_RESULT = res

    out = np.empty((B, HID), dtype=np.float32)
    for c in range(N_CORES):
        out[c * R:(c + 1) * R, :] = res.results[c]["outT"].T
    return out


# revision 13
# speedup vs baseline: 1.0738x; 1.0738x over previous
"""EventDrivenODECell Trainium2 kernel.

Math (reference semantics):
  dt = (t_end - t_start)/5
  5 Euler steps: h += dt * (W3 tanh(W2 tanh(W1a h + [bd1 + W1b te(t)])) + bd3)
    where te(t) depends only on the scalar t -> folded on host into a
    per-step bias  b1s = bd1 + W1b @ te(t_s);  dt folded into W3/bd3.
  event: out = h + sigmoid(Wg ef + bg) * (We2 relu(We1h h + We1e ef + be1) + be2)

Device layout: feature-major activations [feat, batch]; batch sharded 8 ways
(8192 rows/core), processed in 8 column-chunks of 1024 (= 2 PSUM banks per
[128,1024] psum tile; matmuls cover it as two N=512 halves). Matmul inputs
are float32r (full-rate PE mode, ~12-bit mantissa); the h accumulator is
stored f32r (rounding applied by the DVE update op); biases and PSUM stay
fp32. Weight blocks are kept stationary across 2 chunks x 2 N-halves
(4 matmuls per LDWEIGHTS). tanh/sigmoid on ACT (bias fused), relu and all
elementwise adds on DVE (bias fused via scalar_tensor_tensor/tensor_scalar).
"""

import os
import sys

sys.path.insert(0, "/opt/trn_rl_repo")

import numpy as np

import concourse.bacc as bacc
import concourse.mybir as mybir
import concourse.tile as tile
from concourse.bass_utils import run_bass_kernel_spmd

B = 65536
HID = 256
EVT = 64
TEMB = 32
NUM_STEPS = 5
N_CORES = 8
R = B // N_CORES          # rows per core
CHUNK = 1024
N_CHUNKS = R // CHUNK     # 8
GROUP = 2                 # chunks per compute group (2 chunks x 2 m-halves
                          # x 2 banks = all 8 PSUM banks per layer)

MODE = os.environ.get("KMODE", "f32r")   # "f32r" | "f32" | "f16"

f32 = mybir.dt.float32
f32r = mybir.dt.float32r
f16 = mybir.dt.float16

_CACHE = {}

# bias-pack column indices
COL_B1S = 0          # 0..4: per-step layer-1 bias
COL_B2 = 5
COL_B3 = 6
COL_BE1 = 7
COL_BE2 = 8
COL_BG = 9
N_BIAS_COLS = 10


def _build(mode):
    wdt = {"f32r": f32r, "f32": f32, "f16": f16}[mode]
    nc = bacc.Bacc("TRN2", target_bir_lowering=False, debug=False,
                   num_devices=N_CORES)

    hT_d = nc.dram_tensor("hT", [HID, R], wdt, kind="ExternalInput")
    efT_d = nc.dram_tensor("efT", [EVT, R], wdt, kind="ExternalInput")
    w1_d = nc.dram_tensor("w1", [HID, HID], wdt, kind="ExternalInput")
    w2_d = nc.dram_tensor("w2", [HID, HID], wdt, kind="ExternalInput")
    w3_d = nc.dram_tensor("w3", [HID, HID], wdt, kind="ExternalInput")
    we1h_d = nc.dram_tensor("we1h", [HID, HID], wdt, kind="ExternalInput")
    we1e_d = nc.dram_tensor("we1e", [EVT, HID], wdt, kind="ExternalInput")
    we2_d = nc.dram_tensor("we2", [HID, HID], wdt, kind="ExternalInput")
    wg_d = nc.dram_tensor("wg", [EVT, HID], wdt, kind="ExternalInput")
    biasp_d = nc.dram_tensor("biasp", [HID, N_BIAS_COLS], f32,
                             kind="ExternalInput")
    outT_d = nc.dram_tensor("outT", [HID, R], f32, kind="ExternalOutput")

    Tanh = mybir.ActivationFunctionType.Tanh
    Sigmoid = mybir.ActivationFunctionType.Sigmoid
    add = mybir.AluOpType.add
    mult = mybir.AluOpType.mult
    vmax = mybir.AluOpType.max

    with tile.TileContext(nc) as tc:
        with (
            tc.tile_pool(name="consts", bufs=1) as consts,
            tc.tile_pool(name="h", bufs=1) as h_pool,
            tc.tile_pool(name="z1", bufs=2 * GROUP + 2) as z1_pool,
            tc.tile_pool(name="z2", bufs=2 * GROUP + 2) as z2_pool,
            tc.tile_pool(name="efc", bufs=4) as ef_pool,
            tc.tile_pool(name="stage", bufs=6) as stage_pool,
            tc.tile_pool(name="psum", bufs=4, space="PSUM") as psum_pool,
        ):
            # ---- constants ----
            def load_w(d, name, kparts, kdim=128):
                ts = []
                for k in range(kparts):
                    t = consts.tile([kdim, HID], wdt, tag=f"{name}{k}",
                                    name=f"{name}{k}")
                    nc.sync.dma_start(t[:], d.ap()[k * kdim:(k + 1) * kdim, :])
                    ts.append(t)
                return ts

            w1 = load_w(w1_d, "w1", 2)
            w2 = load_w(w2_d, "w2", 2)
            w3 = load_w(w3_d, "w3", 2)
            we1h = load_w(we1h_d, "we1h", 2)
            we2 = load_w(we2_d, "we2", 2)
            we1e = load_w(we1e_d, "we1e", 1, kdim=EVT)[0]   # [64, 256]
            wg = load_w(wg_d, "wg", 1, kdim=EVT)[0]         # [64, 256]

            biasp = []
            for m in range(2):
                t = consts.tile([128, N_BIAS_COLS], f32, tag=f"biasp{m}",
                                name=f"biasp{m}")
                nc.sync.dma_start(t[:], biasp_d.ap()[m * 128:(m + 1) * 128, :])
                biasp.append(t)

            def bcol(m, col):
                return biasp[m][:, col:col + 1]

            # ---- persistent h tiles (f32r in fast mode) ----
            h = [[h_pool.tile([128, CHUNK], wdt, tag=f"h{c}_{m}",
                              name=f"h{c}_{m}")
                  for m in range(2)] for c in range(N_CHUNKS)]
            for c in range(N_CHUNKS):
                for m in range(2):
                    nc.sync.dma_start(
                        h[c][m][:],
                        hT_d.ap()[m * 128:(m + 1) * 128,
                                  c * CHUNK:(c + 1) * CHUNK])

            def matmul_layer(chunks, win, xs, psums, kparts=2, extra=None):
                """psums[c][m] ([128,CHUNK] PSUM) = win.T @ xs[c] (+ extra).

                Weight block (k,m) stays stationary across len(chunks)*2
                N=512 matmuls. extra = (w_tile[EVT,HID], {c: x_ap}) adds an
                EVT-dim contribution.
                """
                n_acc = kparts + (1 if extra is not None else 0)
                for m in range(2):
                    for k in range(kparts):
                        wblk = win[k][:, m * 128:(m + 1) * 128]
                        for c in chunks:
                            for nh in range(CHUNK // 512):
                                nsl = slice(nh * 512, (nh + 1) * 512)
                                nc.tensor.matmul(
                                    psums[c][m][:, nsl],
                                    wblk, xs[c][k][:, nsl],
                                    start=(k == 0), stop=(k == n_acc - 1))
                    if extra is not None:
                        ew, exs = extra
                        eblk = ew[:, m * 128:(m + 1) * 128]
                        for c in chunks:
                            for nh in range(CHUNK // 512):
                                nsl = slice(nh * 512, (nh + 1) * 512)
                                nc.tensor.matmul(
                                    psums[c][m][:, nsl], eblk,
                                    exs[c][:, nsl],
                                    start=False, stop=True)

            def act_layer(chunks, psums, pool, act, bias_col, out_dt):
                outs = {}
                for c in chunks:
                    ts = []
                    for m in range(2):
                        o = pool.tile([128, CHUNK], out_dt, tag="z",
                                      name=f"z{c}_{m}")
                        nc.scalar.activation(o[:], psums[c][m][:], act,
                                             bias=bcol(m, bias_col))
                        ts.append(o)
                    outs[c] = ts
                return outs

            def new_psums(chunks):
                return {c: [psum_pool.tile([128, CHUNK], f32, tag="ps",
                                           name=f"ps{c}_{m}")
                            for m in range(2)] for c in chunks}

            groups = [range(g * GROUP, (g + 1) * GROUP)
                      for g in range(N_CHUNKS // GROUP)]

            n_steps = int(os.environ.get("KSTEPS", NUM_STEPS))
            skip_event = os.environ.get("KSKIP_EVENT", "0") == "1"
            for chunks in groups:
                # ---- ODE: 5 Euler steps, layer-major within the group ----
                for s in range(n_steps):
                    ps1 = new_psums(chunks)
                    matmul_layer(chunks, w1, {c: h[c] for c in chunks}, ps1)
                    z1 = act_layer(chunks, ps1, z1_pool, Tanh,
                                   COL_B1S + s, wdt)
                    ps2 = new_psums(chunks)
                    matmul_layer(chunks, w2, z1, ps2)
                    z2 = act_layer(chunks, ps2, z2_pool, Tanh, COL_B2, wdt)
                    ps3 = new_psums(chunks)
                    matmul_layer(chunks, w3, z2, ps3)
                    for c in chunks:
                        for m in range(2):
                            # h += (psum + b3)  (rounds h to wdt on store)
                            nc.vector.scalar_tensor_tensor(
                                h[c][m][:], ps3[c][m][:], bcol(m, COL_B3),
                                h[c][m][:], op0=add, op1=add)

                # ---- event update for this group ----
                if skip_event:
                    for c in chunks:
                        for m in range(2):
                            stg = stage_pool.tile([128, CHUNK], f32,
                                                  tag="st", name=f"s{c}_{m}")
                            nc.vector.tensor_copy(stg[:], h[c][m][:])
                            nc.sync.dma_start(
                                outT_d.ap()[m * 128:(m + 1) * 128,
                                            c * CHUNK:(c + 1) * CHUNK],
                                stg[:])
                    continue
                efs = {}
                for c in chunks:
                    efc = ef_pool.tile([EVT, CHUNK], wdt, tag="ef",
                                       name=f"ef{c}")
                    nc.sync.dma_start(
                        efc[:], efT_d.ap()[:, c * CHUNK:(c + 1) * CHUNK])
                    efs[c] = efc[:]
                psu = new_psums(chunks)
                matmul_layer(chunks, we1h, {c: h[c] for c in chunks}, psu,
                             extra=(we1e, efs))
                u1 = {}
                for c in chunks:
                    ts = []
                    for m in range(2):
                        o = z1_pool.tile([128, CHUNK], wdt, tag="z",
                                         name=f"u{c}_{m}")
                        # relu(psum + be1) on DVE
                        nc.vector.tensor_scalar(
                            o[:], psu[c][m][:], bcol(m, COL_BE1), 0.0,
                            op0=add, op1=vmax)
                        ts.append(o)
                    u1[c] = ts
                # gate first (own PSUM phase), then upd — keeps each event
                # sub-phase within the 4 psum slots, no cross-phase cycle.
                psg = new_psums(chunks)
                for m in range(2):
                    gblk = wg[:, m * 128:(m + 1) * 128]
                    for c in chunks:
                        for nh in range(CHUNK // 512):
                            nsl = slice(nh * 512, (nh + 1) * 512)
                            nc.tensor.matmul(psg[c][m][:, nsl], gblk,
                                             efs[c][:, nsl],
                                             start=True, stop=True)
                gates = {}
                for c in chunks:
                    ts = []
                    for m in range(2):
                        gate = z2_pool.tile([128, CHUNK], f32, tag="z",
                                            name=f"g{c}_{m}")
                        nc.scalar.activation(gate[:], psg[c][m][:], Sigmoid,
                                             bias=bcol(m, COL_BG))
                        ts.append(gate)
                    gates[c] = ts
                psp = new_psums(chunks)
                matmul_layer(chunks, we2, u1, psp)
                for c in chunks:
                    for m in range(2):
                        # tmp = (psum_upd + be2) * gate
                        tmp = z2_pool.tile([128, CHUNK], f32, tag="z",
                                           name=f"t{c}_{m}")
                        nc.vector.scalar_tensor_tensor(
                            tmp[:], psp[c][m][:], bcol(m, COL_BE2),
                            gates[c][m][:], op0=add, op1=mult)
                        # out = tmp + h
                        stg = stage_pool.tile([128, CHUNK], f32, tag="st",
                                              name=f"s{c}_{m}")
                        nc.vector.tensor_add(stg[:], tmp[:], h[c][m][:])
                        nc.sync.dma_start(
                            outT_d.ap()[m * 128:(m + 1) * 128,
                                        c * CHUNK:(c + 1) * CHUNK],
                            stg[:])

    nc.finalize()
    return nc


def _get_nc(mode):
    if mode not in _CACHE:
        _CACHE[mode] = _build(mode)
    return _CACHE[mode]


LAST_RESULT = None


def kernel(h_prev, event_features, t_start, t_end,
           Wt1, bt1, Wt2, bt2,
           Wd1, bd1, Wd2, bd2, Wd3, bd3,
           We1, be1, We2, be2, Wg, bg):
    global LAST_RESULT
    assert h_prev.shape == (B, HID) and event_features.shape == (B, EVT)

    # ---- host-side folding (float64 for exactness, cast to f32) ----
    f8 = np.float64
    dt = (f8(t_end) - f8(t_start)) / NUM_STEPS
    b1s = np.empty((HID, NUM_STEPS), dtype=f8)
    for s in range(NUM_STEPS):
        t = f8(t_start) + s * dt
        te = np.tanh(t * Wt1[:, 0].astype(f8) + bt1.astype(f8))
        te = Wt2.astype(f8) @ te + bt2.astype(f8)
        b1s[:, s] = bd1.astype(f8) + Wd1[:, HID:].astype(f8) @ te

    xdt = np.float16 if MODE == "f16" else np.float32
    w1T = np.ascontiguousarray(Wd1[:, :HID].T, dtype=xdt)
    w2T = np.ascontiguousarray(Wd2.T, dtype=xdt)
    w3T = np.ascontiguousarray((dt * Wd3.astype(f8)).T.astype(xdt))
    we1hT = np.ascontiguousarray(We1[:, :HID].T, dtype=xdt)
    we1eT = np.ascontiguousarray(We1[:, HID:].T, dtype=xdt)
    we2T = np.ascontiguousarray(We2.T, dtype=xdt)
    wgT = np.ascontiguousarray(Wg.T, dtype=xdt)

    biasp = np.zeros((HID, N_BIAS_COLS), dtype=f8)
    biasp[:, COL_B1S:COL_B1S + NUM_STEPS] = b1s
    biasp[:, COL_B2] = bd2.astype(f8)
    biasp[:, COL_B3] = dt * bd3.astype(f8)
    biasp[:, COL_BE1] = be1.astype(f8)
    biasp[:, COL_BE2] = be2.astype(f8)
    biasp[:, COL_BG] = bg.astype(f8)
    biasp = biasp.astype(np.float32)

    hT = np.ascontiguousarray(h_prev.T, dtype=xdt)      # [HID, B]
    efT = np.ascontiguousarray(event_features.T, dtype=xdt)

    shared = dict(w1=w1T, w2=w2T, w3=w3T, we1h=we1hT, we1e=we1eT,
                  we2=we2T, wg=wgT, biasp=biasp)
    in_maps = []
    for c in range(N_CORES):
        sl = slice(c * R, (c + 1) * R)
        in_maps.append(dict(
            hT=np.ascontiguousarray(hT[:, sl]),
            efT=np.ascontiguousarray(efT[:, sl]),
            **shared))

    nc = _get_nc(MODE)
    res = run_bass_kernel_spmd(nc, in_maps, core_ids=list(range(N_CORES)))
    LAST_RESULT = res

    out = np.empty((B, HID), dtype=np.float32)
    for c in range(N_CORES):
        out[c * R:(c + 1) * R, :] = res.results[c]["outT"].T
    return out
